# revision 2
# baseline (speedup 1.0000x reference)
"""Trainium2 Bass kernel for nn_BCNLayer (locally-connected 7x7 lattice layer + sigmoid).

Math: y[i,j,b] = sigmoid( sum_{dy,dx in [-3,3]} w[dy+3,dx+3][(i-dy)*W + (j-dx)]
                          * x[(i-dy)*W + (j-dx), b] )   (zero outside lattice)

Strategy:
  - 8-way shard over lattice rows (H=128 -> 16 dest rows/core, 22 source rows
    with 3-row halos, zero-padded at the edges).
  - For one dest row i and source-row offset d (7 of them), the contribution is
    a banded 128x128 matrix (band +-3 over lattice columns) applied to the
    source row's [128 cols x B batch] slab:  out[jd, b] += sum_js
    Wband[js, jd] * x[js, b].  That is exactly nc.tensor.matmul(psum, lhsT=Wband,
    rhs=xrow) accumulated over the 7 source rows.
  - Banded matrices are prebuilt on the host (numpy) and DMA'd in; HW executes
    pure DMA + matmul + sigmoid.
  - DMA plan: x streams on the SP HWDGE ring in fine-grained pieces (chunk-0
    rows first, so the first matmul starts ~5us earlier); the banded weights
    stream on the ACT HWDGE ring in partition-major layout (>=1.75KB contiguous
    runs per descriptor, line-rate) with a tiny [t0,d0] piece first so the
    first LDWEIGHTS is never the gating input.  Output DMAs go on the gpsimd
    (SWDGE) ring, which is otherwise idle.
"""

import os

import numpy as np

H = 128
W = 128
HW = H * W
B = 1024
NCORES = 8
T = H // NCORES  # dest rows per core = 16
S = T + 6        # source rows per core (halo 3 each side) = 22
BC = 512         # batch chunk (fp32 psum bank = 512 fp32 per partition)
NB = B // BC     # chunks = 2

# dtype mode for the matmul inputs:
#   "f16"  - fp16 x and weights (10-bit mantissa, halves input traffic; fast)
#   "f32r" - tf32 path (10-bit mantissa products, fp32-sized traffic)
#   "f32"  - exact fp32 (4x slower matmul)
MM_MODE = os.environ.get("KERNEL_MM_MODE", "f16")
# output dtype: bf16 halves output traffic; adds <=2^-9 relative rounding
# (tolerance is 2e-2; f16 compute already sits at ~8e-3)
OUT_MODE = os.environ.get("KERNEL_OUT_MODE", "bf16")

_cache: dict = {}

# filled by the last kernel() call when KERNEL_TRACE=1
last_exec_time_ns = None
last_results = None

# x chunk-0 pieces: fine-grained at the front so matmul (t=0, d) can start as
# soon as source row d lands; chunk-1 in coarse pieces (needed ~20us later).
X_PIECES_C0 = [(0, 1), (1, 3), (3, 7), (7, 11), (11, 15), (15, 19), (19, 22)]
X_PIECES_C1 = [(0, 7), (7, 15), (15, 22)]
# wb pieces in units of 128x128 matrices (t*7+d flat index): [t0,d0] first.
WB_PIECES = [(0, 1), (1, 7), (7, 28), (28, 70), (70, 112)]


def _build_program(mode: str, out_mode: str):
    from contextlib import ExitStack

    import concourse.bacc as bacc
    import concourse.mybir as mybir
    import concourse.tile as tile

    nc = bacc.Bacc(
        "TRN2", target_bir_lowering=False, debug=False, num_devices=NCORES
    )
    mm_dt = {
        "f32": mybir.dt.float32,
        "f32r": mybir.dt.float32r,
        "f16": mybir.dt.float16,
    }[mode]
    out_dt = {
        "f32": mybir.dt.float32,
        "bf16": mybir.dt.bfloat16,
    }[out_mode]
    xs = nc.dram_tensor("xs", [S, 128, B], mm_dt, kind="ExternalInput").ap()
    # p-major banded weights: [js (partition), t*7*128 flat (t, d, jd)]
    wb = nc.dram_tensor(
        "wb", [128, T * 7 * 128], mm_dt, kind="ExternalInput"
    ).ap()
    y = nc.dram_tensor(
        "y", [T, 128, B], out_dt, kind="ExternalOutput"
    ).ap()

    with tile.TileContext(nc) as tc, ExitStack() as ctx:
        xpool = ctx.enter_context(tc.tile_pool(name="x", bufs=1))
        wpool = ctx.enter_context(tc.tile_pool(name="w", bufs=1))
        ppool = ctx.enter_context(tc.tile_pool(name="ps", bufs=8, space="PSUM"))
        opool = ctx.enter_context(tc.tile_pool(name="o", bufs=6))

        xt = xpool.tile([128, S * B], mm_dt, tag="xslab")
        wt = wpool.tile([128, T * 7 * 128], mm_dt, tag="wslab")

        # Warm the sigmoid ACT table during the load phase (it otherwise loads
        # lazily right before the first real sigmoid, stalling the pipeline).
        warm = opool.tile([128, 1], mybir.dt.float32, tag="warm")
        nc.vector.memset(warm[:], 0.0)
        nc.scalar.activation(warm[:], warm[:], mybir.ActivationFunctionType.Sigmoid)

        xt3 = xt[:].rearrange("p (s b) -> p s b", s=S)

        # wb pieces on the ACT HWDGE ring (issued by the scalar engine).
        for lo, hi in WB_PIECES:
            nc.scalar.dma_start(
                out=wt[:, lo * 128 : hi * 128], in_=wb[:, lo * 128 : hi * 128]
            )

        # x pieces on the SP HWDGE ring, chunk 0 first.
        for c, pieces in ((0, X_PIECES_C0), (1, X_PIECES_C1)):
            for lo, hi in pieces:
                nc.sync.dma_start(
                    out=xt3[:, lo:hi, c * BC : (c + 1) * BC],
                    in_=xs[lo:hi, :, c * BC : (c + 1) * BC].rearrange(
                        "s p b -> p s b"
                    ),
                )

        for c in range(NB):
            for t in range(T):
                ps = ppool.tile([128, BC], mybir.dt.float32, tag="ps")
                for d in range(7):
                    lhs = wt[:, (t * 7 + d) * 128 : (t * 7 + d + 1) * 128]
                    rhs = xt[:, (t + d) * B + c * BC : (t + d) * B + (c + 1) * BC]
                    nc.tensor.matmul(
                        ps[:], lhs, rhs, start=(d == 0), stop=(d == 6)
                    )
                ot = opool.tile([128, BC], out_dt, tag="o")
                nc.scalar.activation(
                    ot[:], ps[:], mybir.ActivationFunctionType.Sigmoid
                )
                nc.gpsimd.dma_start(
                    out=y[t, :, c * BC : (c + 1) * BC], in_=ot[:]
                )
    nc.compile()
    return nc


def _build_banded(weights: np.ndarray) -> np.ndarray:
    """G[i, d, js, jd] = weight of edge (src row i+d-3, col js) -> (dest row i, col jd).

    dy = 3 - d (dest = src + dy), dx = jd - js, weight index = w[dy+3, dx+3][src_hw].
    """
    w4 = weights.reshape(7, 7, H, W)
    G = np.zeros((H, 7, W, W), np.float32)
    i = np.arange(H)
    for d in range(7):
        r = i + d - 3
        vi = i[(r >= 0) & (r < H)]
        if len(vi) == 0:
            continue
        for dxi in range(7):
            dx = dxi - 3
            js = np.arange(max(0, -dx), W - max(0, dx))
            G[vi[:, None], d, js[None, :], js[None, :] + dx] = w4[6 - d, dxi][
                (vi + d - 3)[:, None], js[None, :]
            ]
    return G


def kernel(x: np.ndarray, weights: np.ndarray) -> np.ndarray:
    global last_exec_time_ns, last_results
    import ml_dtypes
    from concourse.bass_utils import run_bass_kernel_spmd

    x = np.ascontiguousarray(x, dtype=np.float32)
    weights = np.ascontiguousarray(weights, dtype=np.float32)

    key = (MM_MODE, OUT_MODE)
    if key not in _cache:
        _cache[key] = _build_program(MM_MODE, OUT_MODE)
    nc = _cache[key]

    io_dt = np.float16 if MM_MODE == "f16" else np.float32
    x3 = x.reshape(H, W, B)
    xp = np.zeros((H + 6, W, B), io_dt)
    xp[3 : H + 3] = x3.astype(io_dt)
    G = _build_banded(weights).astype(io_dt)

    in_maps = []
    for q in range(NCORES):
        # [t, d, js, jd] -> partition-major [js, t, d, jd]
        Gq = G[T * q : T * q + T].transpose(2, 0, 1, 3)
        in_maps.append(
            {
                "xs": np.ascontiguousarray(xp[T * q : T * q + S]),
                "wb": np.ascontiguousarray(Gq.reshape(W, T * 7 * W)),
            }
        )

    trace = os.environ.get("KERNEL_TRACE", "0") == "1"
    res = run_bass_kernel_spmd(
        nc, in_maps, core_ids=list(range(NCORES)), trace=trace
    )
    last_exec_time_ns = res.exec_time_ns
    last_results = res
    out = np.concatenate(
        [np.asarray(r["y"]).astype(np.float32).reshape(T * W, B) for r in res.results],
        axis=0,
    )
    return out
